# revision 1
# baseline (speedup 1.0000x reference)
"""Trainium2 Bass kernel for nn_MultiHeadSelfAttention (N=2, S=2048, E=1024, H=16).

Sharding: heads+batch tensor-parallel over 8 cores. Core c handles batch
n = c // 4 and 4 heads h in [4*(c%4), 4*(c%4)+4). Attention is computed
per-head in a transposed layout (contraction dims on SBUF partitions);
fc_out is row-parallel: each core computes a partial y over its 256
embedding dims, and the host sums 4 partials per batch and adds bias.

Per-core pipeline (all matmuls fp32r = full PE rate at N=512):
  energyT[k, q] = KT.T @ QT             (lhsT = KT chunk, rhs = QT slice)
  attT = exp(energyT / 32)              (ScalarE activation, PSUM -> SBUF)
  out2T/den = [V | ones].T @ attT       (ones column makes the softmax
                                         denominator fall out as row 64)
  X^T = out2T * (1/den) broadcast       (DMA partition-broadcast + DVE)
  y_partial = X^T.T @ W^T               (accumulate 4 heads in PSUM)
"""

import time

import numpy as np

N_CORES = 8
NB = 2          # batch
S = 2048        # sequence length
E = 1024        # embed size
H = 16          # heads
D = 64          # head dim
HPC = 4         # heads per core
SCALE = float(1.0 / np.sqrt(E))  # softmax scale (embed_size**0.5)

KC = S // 128   # 16 contraction chunks of 128 keys
QB = S // 512   # 4 query blocks of 512
EG = 2          # energy chunks per psum group ([128, 1024] tiles)

# exp(x) ~= p(t)^8, p monic cubic, t = x pre-scaled by S_Q on the host.
# ScalarE chunks use exp(t * ACT_SCALE) (exact); VectorE chunks use the
# 8-stage custom DVE polynomial (rel err ~2.6e-4).
ALPHA = 6.0 ** (1.0 / 3.0)
S_Q = float(SCALE / (8.0 * ALPHA))   # host pre-scale on Q (t = S_Q/SCALE * x)
ACT_SCALE = float(8.0 * ALPHA)
B2, B1, B0 = 1.6574587989430332, 1.8171403999384372, 0.9999891634709047


def register_exp_op():
    """Register the EXP_POLY8_ANT custom DVE op (idempotent)."""
    import concourse.dve_ops as dve_ops
    from concourse.dve_ops import OPS, DveOp
    from concourse.dve_spec import C0, C1, C2, Spec, Src0, _has_src1, lower, sq
    from concourse.dve_uop import DveOpSpec

    name = "EXP_POLY8_ANT"
    for op in OPS:
        if op.name == name:
            return op

    _p = ((Src0 + C0) * Src0 + C1) * Src0 + C2
    _body = sq(sq(sq(_p)))

    def _ref(in0, in1, s0, s1, imm2):
        p = ((in0 + s0) * in0 + s1) * in0 + imm2
        return ((p ** 2) ** 2) ** 2

    spec = Spec(body=_body, reference=_ref)
    opcode = dve_ops._CUSTOM_DVE_ROW_BASE + len(OPS)
    shas = {}
    for ver in ("v3", "v4"):
        d = DveOpSpec(
            name=name, opcode=opcode, uops=lower(spec, ver=ver),
            rd1_en=_has_src1(spec),
        )
        shas[ver] = d.sha(ver)
    op = DveOp(name, spec, subdim=False, uops_sha=shas)
    OPS.append(op)
    dve_ops._SUB_OPCODE_FOR_NAME[name] = opcode
    dve_ops.CUSTOM_DVE_SPECS[name] = spec
    return op


def build_kernel(reps=1, dve_exp_mod=3, abufs=4, ybufs=4):
    import contextlib

    import concourse.bacc as bacc
    import concourse.bass as bass
    import concourse.tile as tile
    from concourse import mybir

    F32 = mybir.dt.float32
    F32R = mybir.dt.float32r

    exp_op = register_exp_op()

    nc = bacc.Bacc("TRN2", target_bir_lowering=False, num_devices=N_CORES)

    qt = nc.dram_tensor("qt", [HPC, D, S], F32R, kind="ExternalInput")
    kt = nc.dram_tensor("kt", [HPC, D, S], F32R, kind="ExternalInput")
    vb = nc.dram_tensor("vb", [HPC, S, D + 1], F32R, kind="ExternalInput")
    wt = nc.dram_tensor("wt", [HPC, D, E], F32R, kind="ExternalInput")
    tok = nc.dram_tensor("tok", [1, 128], F32, kind="ExternalInput")
    yp = nc.dram_tensor("yp", [S, E], F32, kind="ExternalOutput")
    tok_out = nc.dram_tensor("tok_out", [1, 128], F32, kind="ExternalOutput")

    with tile.TileContext(nc) as tc:
        with contextlib.ExitStack() as ctx:
            singles = ctx.enter_context(tc.tile_pool(name="singles", bufs=1))
            vpool = ctx.enter_context(tc.tile_pool(name="vpool", bufs=2))
            epool = ctx.enter_context(
                tc.tile_pool(name="epool", bufs=2, space="PSUM")
            )
            opool = ctx.enter_context(
                tc.tile_pool(name="opool", bufs=2, space="PSUM")
            )
            ypool = ctx.enter_context(
                tc.tile_pool(name="ypool", bufs=2, space="PSUM")
            )
            apool = ctx.enter_context(tc.tile_pool(name="apool", bufs=abufs))
            npool = ctx.enter_context(tc.tile_pool(name="npool", bufs=2))
            ysb_pool = ctx.enter_context(tc.tile_pool(name="ysb", bufs=ybufs))

            # token passthrough for timing chains
            tok_sb = singles.tile([1, 128], F32)
            nc.gpsimd.dma_start(out=tok_sb, in_=tok[:, :])
            nc.gpsimd.dma_start(out=tok_out[:, :], in_=tok_sb)

            # resident inputs
            qt_sb, kt_sb, wt_sb, xt_sb = [], [], [], []
            for h in range(HPC):
                q_t = singles.tile([D, S], F32R, tag=f"qt{h}")
                nc.sync.dma_start(out=q_t, in_=qt[h])
                qt_sb.append(q_t)
                k_t = singles.tile([D, S], F32R, tag=f"kt{h}")
                nc.sync.dma_start(out=k_t, in_=kt[h])
                kt_sb.append(k_t)
                w_t = singles.tile([D, E], F32R, tag=f"wt{h}")
                nc.sync.dma_start(out=w_t, in_=wt[h])
                wt_sb.append(w_t)
                x_t = singles.tile([D, S], F32R, tag=f"xt{h}")
                xt_sb.append(x_t)

            loop_cm = tc.For_i(0, reps, 1) if reps > 1 else contextlib.nullcontext()
            ctx.enter_context(loop_cm)

            exp_idx = 0
            pend_att = None   # deferred attV emission (one group behind)
            pend_norm = None  # deferred normalization (one (h, qb) behind)

            def make_att(v_t, a_t, o_t, g):
                def emit():
                    for j in range(EG):
                        kchunk = g * EG + j
                        nc.tensor.matmul(
                            o_t,
                            lhsT=(v_t[:, kchunk, :]),
                            rhs=(a_t[:, j * 512 : (j + 1) * 512]),
                            start=(kchunk == 0),
                            stop=(kchunk == KC - 1),
                        )
                return emit

            def make_norm(h, qb, o_t):
                def emit():
                    # normalization: den is row D (=64) of o_t
                    den_sb = npool.tile([D + 1, 512], F32, tag="den")
                    nc.scalar.copy(
                        out=den_sb[D : D + 1, :], in_=o_t[D : D + 1, :]
                    )
                    den_row = den_sb[D : D + 1, :]
                    bcast = npool.tile([D, 512], F32, tag="bcast")
                    # replicate the den row 64x: step-0 on a free dim
                    # (partition dims need nonzero step), written
                    # partition-major into bcast[64, 512]
                    den_b = bass.AP(
                        tensor=den_row.tensor,
                        offset=den_row.offset,
                        ap=[list(den_row.ap[0]), [0, D]]
                        + [list(x) for x in den_row.ap[1:]],
                    )
                    nc.sync.dma_start(out=bcast, in_=den_b)
                    rec = npool.tile([D, 512], F32, tag="rec")
                    nc.vector.reciprocal_approx_fast(out=rec, in_=bcast)
                    nc.vector.tensor_mul(
                        out=xt_sb[h][:, qb * 512 : (qb + 1) * 512],
                        in0=o_t[0:D, :],
                        in1=rec,
                    )
                return emit

            for h in range(HPC):
                # V tiles for this head: [128, kc, 65]
                v_t = vpool.tile([128, KC, D + 1], F32R)
                nc.sync.dma_start(
                    out=v_t, in_=vb[h].rearrange("(kc p) c -> p kc c", p=128)
                )
                for qb in range(QB):
                    qs = qt_sb[h][:, qb * 512 : (qb + 1) * 512]
                    o_t = opool.tile([D + 1, 512], F32)
                    for g in range(KC // EG):
                        e_t = epool.tile([128, EG * 512], F32)
                        for j in range(EG):
                            kchunk = g * EG + j
                            nc.tensor.matmul(
                                e_t[:, j * 512 : (j + 1) * 512],
                                lhsT=(
                                    kt_sb[h][:, kchunk * 128 : (kchunk + 1) * 128]
                                ),
                                rhs=(qs),
                                start=True,
                                stop=True,
                            )
                        if g == 1 and pend_norm is not None:
                            pend_norm()
                            pend_norm = None
                        a_t = apool.tile([128, EG * 512], F32R)
                        if (
                            dve_exp_mod == 0
                            or exp_idx % dve_exp_mod != dve_exp_mod - 1
                        ):
                            nc.scalar.activation(
                                out=a_t,
                                in_=e_t,
                                func=mybir.ActivationFunctionType.Exp,
                                scale=ACT_SCALE,
                            )
                        else:
                            nc.vector._custom_dve(
                                exp_op, out=a_t, in0=e_t, s0=B2, s1=B1, imm2=B0
                            )
                        exp_idx += 1
                        if pend_att is not None:
                            pend_att()
                        pend_att = make_att(v_t, a_t, o_t, g)
                    pend_norm_prev = pend_norm
                    pend_norm = None
                    assert pend_norm_prev is None
                    pend_norm = make_norm(h, qb, o_t)
            # flush tail
            if pend_att is not None:
                pend_att()
                pend_att = None
            if pend_norm is not None:
                pend_norm()
                pend_norm = None

            # fc phase: y[q, f] partial over this core's 4 heads
            for q128 in range(S // 128):
                for f in range(E // 512):
                    y_t = ypool.tile([128, 512], F32)
                    for h in range(HPC):
                        nc.tensor.matmul(
                            y_t,
                            lhsT=(xt_sb[h][:, q128 * 128 : (q128 + 1) * 128]),
                            rhs=(wt_sb[h][:, f * 512 : (f + 1) * 512]),
                            start=(h == 0),
                            stop=(h == HPC - 1),
                        )
                    y_sb = ysb_pool.tile([128, 512], F32)
                    if (q128 * 2 + f) % 2 == 0:
                        nc.scalar.copy(out=y_sb, in_=y_t)
                    else:
                        nc.vector.tensor_copy(y_sb, y_t)
                    nc.sync.dma_start(
                        out=yp[
                            q128 * 128 : (q128 + 1) * 128, f * 512 : (f + 1) * 512
                        ],
                        in_=y_sb,
                    )
    nc.compile()
    return nc


class SpmdRunner:
    """Build one jitted shard_map callable over 8 cores; reusable for timing."""

    def __init__(self, nc, n_cores):
        import jax
        from jax.experimental.shard_map import shard_map
        from jax.sharding import Mesh, PartitionSpec

        from concourse import mybir
        from concourse.bass2jax import _bass_exec_p, install_neuronx_cc_hook
        from concourse.bass2jax import partition_id_tensor as _pid

        install_neuronx_cc_hook()
        self.jax = jax
        self.nc = nc
        self.n_cores = n_cores
        self.PartitionSpec = PartitionSpec

        partition_name = nc.partition_id_tensor.name if nc.partition_id_tensor else None
        in_names, out_names, out_avals = [], [], []
        for alloc in nc.m.functions[0].allocations:
            if not isinstance(alloc, mybir.MemoryLocationSet):
                continue
            name = alloc.memorylocations[0].name
            if alloc.kind == "ExternalInput":
                if name != partition_name:
                    in_names.append(name)
            elif alloc.kind == "ExternalOutput":
                out_names.append(name)
                shape = tuple(alloc.tensor_shape)
                dtype = mybir.dt.np(alloc.dtype)
                out_avals.append(jax.core.ShapedArray(shape, dtype))
        self.in_names = in_names
        self.out_names = out_names
        self.out_avals = out_avals
        n_params = len(in_names)
        n_outs = len(out_avals)

        all_in_names = list(in_names) + list(out_names)
        if partition_name is not None:
            all_in_names.append(partition_name)

        def _body(*args):
            operands = list(args)
            if partition_name is not None:
                operands.append(_pid())
            outs = _bass_exec_p.bind(
                *operands,
                out_avals=tuple(out_avals),
                in_names=tuple(all_in_names),
                out_names=tuple(out_names),
                lowering_input_output_aliases=(),
                sim_require_finite=True,
                sim_require_nnan=True,
                nc=nc,
            )
            return tuple(outs)

        self._body = _body
        devices = jax.devices()[:n_cores]
        assert len(devices) == n_cores
        self.mesh = Mesh(np.asarray(devices), ("core",))
        in_specs = (PartitionSpec("core"),) * (n_params + n_outs)
        out_specs = (PartitionSpec("core"),) * n_outs
        self.fn = jax.jit(
            shard_map(
                _body,
                mesh=self.mesh,
                in_specs=in_specs,
                out_specs=out_specs,
                check_rep=False,
            ),
            keep_unused=True,
        )
        self._chain_fns = {}

    def prepare(self, in_maps):
        jax = self.jax
        n = self.n_cores
        concat_in = [
            np.concatenate([np.asarray(in_maps[c][name]) for c in range(n)], axis=0)
            for name in self.in_names
        ]
        concat_zeros = [
            np.zeros((n * a.shape[0], *a.shape[1:]), a.dtype) for a in self.out_avals
        ]
        sharding = jax.sharding.NamedSharding(self.mesh, self.PartitionSpec("core"))
        self.dev_args = [jax.device_put(a, sharding) for a in concat_in + concat_zeros]
        return self.dev_args

    def run(self):
        outs = self.fn(*self.dev_args)
        self.jax.block_until_ready(outs)
        return outs

    def results(self, outs):
        n = self.n_cores
        res = []
        for c in range(n):
            d = {}
            for i, name in enumerate(self.out_names):
                a = np.asarray(outs[i])
                d[name] = a.reshape(n, *self.out_avals[i].shape)[c]
            res.append(d)
        return res

    # ---- timing support: chain K invocations through the tok tensor ----
    def chain_fn(self, k):
        if k in self._chain_fns:
            return self._chain_fns[k]
        jax = self.jax
        from jax.experimental.shard_map import shard_map

        tok_in_idx = self.in_names.index("tok")
        tok_out_idx = self.out_names.index("tok_out")
        n_params = len(self.in_names)

        def _chained(*args):
            args = list(args)
            outs = None
            for _ in range(k):
                outs = self._body(*args)
                args[tok_in_idx] = outs[tok_out_idx]
            return tuple(outs)

        in_specs = (self.PartitionSpec("core"),) * (n_params + len(self.out_names))
        out_specs = (self.PartitionSpec("core"),) * len(self.out_names)
        fn = jax.jit(
            shard_map(
                _chained,
                mesh=self.mesh,
                in_specs=in_specs,
                out_specs=out_specs,
                check_rep=False,
            ),
            keep_unused=True,
        )
        self._chain_fns[k] = fn
        return fn

    def time_chain(self, k, iters=8, warmup=2):
        fn = self.chain_fn(k)
        for _ in range(warmup):
            self.jax.block_until_ready(fn(*self.dev_args))
        ts = []
        for _ in range(iters):
            t0 = time.perf_counter()
            self.jax.block_until_ready(fn(*self.dev_args))
            ts.append(time.perf_counter() - t0)
        return min(ts)


def shard_inputs(values, keys, query, W_out):
    """Build the 8 per-core input maps (host-side layout prep)."""
    v4 = np.asarray(values, np.float32).reshape(NB, S, H, D)
    k4 = np.asarray(keys, np.float32).reshape(NB, S, H, D)
    q4 = np.asarray(query, np.float32).reshape(NB, S, H, D)
    W_out = np.asarray(W_out, np.float32)
    in_maps = []
    tok = np.zeros((1, 128), np.float32)
    for c in range(N_CORES):
        n = c // 4
        h0 = HPC * (c % 4)
        qt = np.ascontiguousarray(
            q4[n, :, h0 : h0 + HPC, :].transpose(1, 2, 0) * np.float32(S_Q)
        )  # [HPC, D, S], pre-scaled so energy arrives as t (see exp notes)
        kt = np.ascontiguousarray(k4[n, :, h0 : h0 + HPC, :].transpose(1, 2, 0))
        vb = np.concatenate(
            [
                np.ascontiguousarray(v4[n, :, h0 : h0 + HPC, :].transpose(1, 0, 2)),
                np.ones((HPC, S, 1), np.float32),
            ],
            axis=2,
        )  # [HPC, S, D+1]
        wt = np.ascontiguousarray(
            W_out[:, (h0 * D) : (h0 + HPC) * D].T.reshape(HPC, D, E)
        )
        in_maps.append({"qt": qt, "kt": kt, "vb": vb, "wt": wt, "tok": tok})
    return in_maps


_CACHE = {}


def get_runner():
    if "runner" not in _CACHE:
        nc = build_kernel()
        _CACHE["runner"] = SpmdRunner(nc, N_CORES)
    return _CACHE["runner"]


def kernel(values, keys, query, W_out, b_out):
    runner = get_runner()
    in_maps = shard_inputs(values, keys, query, W_out)
    runner.prepare(in_maps)
    outs = runner.run()
    res = runner.results(outs)
    y = np.zeros((NB, S, E), np.float32)
    for c in range(N_CORES):
        y[c // 4] += res[c]["yp"]
    y += np.asarray(b_out, np.float32)[None, None, :]
    return y



# revision 26
# speedup vs baseline: 3.1426x; 3.1426x over previous
"""Trainium2 Bass kernel for nn_MultiHeadSelfAttention (N=2, S=2048, E=1024, H=16).

Sharding: heads+batch tensor-parallel over 8 cores. Core c handles batch
n = c // 4 and 4 heads h in [4*(c%4), 4*(c%4)+4), organized as 2 pairs.
Attention is computed per-head in a transposed layout (contraction dims
on SBUF partitions); fc_out is row-parallel: each core computes a partial
y over its 256 embedding dims, and the host sums 4 partials per batch and
adds bias.

v2 (vs fp32r baseline): all PE operands bf16 (fp32 PSUM accumulation),
QK^T row-tiled in head pairs (two K=64 matmuls run concurrently in the
PE array via base-partition-derived tile_position), fc contracts over
pair-stacked K=128, normalization reciprocal reads the denominator row
straight from PSUM.

Per-core pipeline per (pair, qblock):
  for g in 16 key chunks:
    e[:, 0:512]   = KT_a[g].T @ QT_a    (rows 0-63 of the PE array)
    e[:, 512:1024]= KT_b[g].T @ QT_b    (rows 64-127, concurrent)
    a = exp(e / 32)                     (ScalarE Exp / VectorE poly8, bf16)
    oA += [V_a[g] | ones].T @ a[:, 0:512]    (ones col -> den in row 64)
    oB += [V_b[g] | ones].T @ a[:, 512:1024]
  rec = 1/den (DVE, from PSUM row 64); DMA-broadcast; x = o * rec -> xt
fc: y[q,f] = sum_p xt_p[:, q].T @ W_p  (K=128 per pair, 2-mm accumulate)
"""

import time

import numpy as np

N_CORES = 8
NB = 2          # batch
S = 2048        # sequence length
E = 1024        # embed size
H = 16          # heads
D = 64          # head dim
HPC = 4         # heads per core
NP = 2          # head pairs per core
SCALE = float(1.0 / np.sqrt(E))  # softmax scale (embed_size**0.5)

KC = S // 128   # 16 contraction chunks of 128 keys
QB = S // 512   # 4 query blocks of 512

# exp(x) ~= p(t)^8, p monic cubic, t = x pre-scaled by S_Q on the host.
# ScalarE chunks use exp(t * ACT_SCALE) (exact); VectorE chunks use the
# 8-stage custom DVE polynomial (rel err ~2.6e-4).
ALPHA = 6.0 ** (1.0 / 3.0)
S_Q = float(SCALE / (8.0 * ALPHA))   # host pre-scale on Q (t = S_Q/SCALE * x)
ACT_SCALE = float(8.0 * ALPHA)
B2, B1, B0 = 1.6574587989430332, 1.8171403999384372, 0.9999891634709047

# exp engine split: DVE takes these slots of each 16-slot cycle (7/16)
DVE_SLOTS = frozenset((1, 3, 5, 7, 9, 11, 13))


def register_exp_op():
    """Register the EXP_POLY8_ANT custom DVE op (idempotent)."""
    import concourse.dve_ops as dve_ops
    from concourse.dve_ops import OPS, DveOp
    from concourse.dve_spec import C0, C1, C2, Spec, Src0, _has_src1, lower, sq
    from concourse.dve_uop import DveOpSpec

    name = "EXP_POLY8_ANT"
    for op in OPS:
        if op.name == name:
            return op

    _p = ((Src0 + C0) * Src0 + C1) * Src0 + C2
    _body = sq(sq(sq(_p)))

    def _ref(in0, in1, s0, s1, imm2):
        p = ((in0 + s0) * in0 + s1) * in0 + imm2
        return ((p ** 2) ** 2) ** 2

    spec = Spec(body=_body, reference=_ref)
    opcode = dve_ops._CUSTOM_DVE_ROW_BASE + len(OPS)
    shas = {}
    for ver in ("v3", "v4"):
        d = DveOpSpec(
            name=name, opcode=opcode, uops=lower(spec, ver=ver),
            rd1_en=_has_src1(spec),
        )
        shas[ver] = d.sha(ver)
    op = DveOp(name, spec, subdim=False, uops_sha=shas)
    OPS.append(op)
    dve_ops._SUB_OPCODE_FOR_NAME[name] = opcode
    dve_ops.CUSTOM_DVE_SPECS[name] = spec
    return op


def build_kernel(
    reps=1, dve_slots=DVE_SLOTS, abufs=4, ybufs=4, pair_qk=True, debug_x=False
):
    import contextlib

    import concourse.bacc as bacc
    import concourse.bass as bass
    import concourse.tile as tile
    from concourse import mybir

    F32 = mybir.dt.float32
    BF16 = mybir.dt.bfloat16

    exp_op = register_exp_op()

    nc = bacc.Bacc("TRN2", target_bir_lowering=False, num_devices=N_CORES)

    # pair-stacked inputs: rows 0-63 head 2p, rows 64-127 head 2p+1
    qt = nc.dram_tensor("qt", [NP, 128, S], BF16, kind="ExternalInput")
    kt = nc.dram_tensor("kt", [NP, 128, S], BF16, kind="ExternalInput")
    vb = nc.dram_tensor("vb", [HPC, S, D + 1], BF16, kind="ExternalInput")
    wt = nc.dram_tensor("wt", [NP, 128, E], BF16, kind="ExternalInput")
    tok = nc.dram_tensor("tok", [1, 128], F32, kind="ExternalInput")
    yp = nc.dram_tensor("yp", [S, E], BF16, kind="ExternalOutput")
    tok_out = nc.dram_tensor("tok_out", [1, 128], F32, kind="ExternalOutput")
    if debug_x:
        xdbg = nc.dram_tensor("xdbg", [NP * 128, S], BF16, kind="ExternalOutput")
        edbg = nc.dram_tensor("edbg", [128, 1024], F32, kind="ExternalOutput")
        adbg = nc.dram_tensor("adbg", [128, 1024], BF16, kind="ExternalOutput")
        odbg = nc.dram_tensor("odbg", [D + 1, 512], F32, kind="ExternalOutput")
        bdbg = nc.dram_tensor("bdbg", [D, 512], F32, kind="ExternalOutput")
        rdbg = nc.dram_tensor("rdbg", [1, 512], F32, kind="ExternalOutput")

    with tile.TileContext(nc) as tc:
        with contextlib.ExitStack() as ctx:
            singles = ctx.enter_context(tc.tile_pool(name="singles", bufs=1))
            epool = ctx.enter_context(
                tc.tile_pool(name="epool", bufs=2, space="PSUM")
            )
            opool = ctx.enter_context(
                tc.tile_pool(name="opool", bufs=4, space="PSUM")
            )
            apool = ctx.enter_context(tc.tile_pool(name="apool", bufs=abufs))
            npool = ctx.enter_context(tc.tile_pool(name="npool", bufs=4))
            ysb_pool = ctx.enter_context(tc.tile_pool(name="ysb", bufs=ybufs))

            # token passthrough for timing chains
            tok_sb = singles.tile([1, 128], F32)
            nc.gpsimd.dma_start(out=tok_sb, in_=tok[:, :])
            nc.gpsimd.dma_start(out=tok_out[:, :], in_=tok_sb)

            # resident inputs
            qt_sb, kt_sb, wt_sb, xt_sb, v_sb = [], [], [], [], []
            qh_sb, kh_sb = [], []
            for p in range(NP):
                q_t = singles.tile([128, S], BF16, tag=f"qt{p}")
                nc.sync.dma_start(out=q_t, in_=qt[p])
                qt_sb.append(q_t)
                k_t = singles.tile([128, S], BF16, tag=f"kt{p}")
                nc.sync.dma_start(out=k_t, in_=kt[p])
                kt_sb.append(k_t)
                if not pair_qk:
                    for hh in range(2):
                        q_h = singles.tile([D, S], BF16, tag=f"qh{p}_{hh}")
                        nc.sync.dma_start(
                            out=q_h, in_=qt[p][hh * D : (hh + 1) * D, :]
                        )
                        qh_sb.append(q_h)
                        k_h = singles.tile([D, S], BF16, tag=f"kh{p}_{hh}")
                        nc.sync.dma_start(
                            out=k_h, in_=kt[p][hh * D : (hh + 1) * D, :]
                        )
                        kh_sb.append(k_h)
                w_t = singles.tile([128, E], BF16, tag=f"wt{p}")
                nc.sync.dma_start(out=w_t, in_=wt[p])
                wt_sb.append(w_t)
                x_t = singles.tile([128, S], BF16, tag=f"xt{p}")
                xt_sb.append(x_t)
            xh_sb = []
            for h in range(HPC):
                xh = singles.tile([D, S], BF16, tag=f"xh{h}")
                xh_sb.append(xh)
            for h in range(HPC):
                v_t = singles.tile([128, KC, D + 1], BF16, tag=f"v{h}")
                nc.sync.dma_start(
                    out=v_t, in_=vb[h].rearrange("(kc p) c -> p kc c", p=128)
                )
                v_sb.append(v_t)

            loop_cm = tc.For_i(0, reps, 1) if reps > 1 else contextlib.nullcontext()
            ctx.enter_context(loop_cm)

            exp_idx = 0
            pend_att = None    # deferred attV emission (one group behind)
            pend_recip = None  # deferred reciprocal (into next superstep)
            pend_mul = None    # deferred normalize-multiply

            def make_att(p, a_t, o_a, o_b, g):
                def emit():
                    nc.tensor.matmul(
                        o_a,
                        lhsT=(v_sb[2 * p][:, g, :]),
                        rhs=(a_t[:, 0:512]),
                        start=(g == 0),
                        stop=(g == KC - 1),
                    )
                    nc.tensor.matmul(
                        o_b,
                        lhsT=(v_sb[2 * p + 1][:, g, :]),
                        rhs=(a_t[:, 512:1024]),
                        start=(g == 0),
                        stop=(g == KC - 1),
                    )
                return emit

            def make_recip(o_a, o_b, box, rbox=None):
                def emit():
                    for i, o_t in enumerate((o_a, o_b)):
                        # lane-aligned den copy (partition 64 -> 64), then
                        # DMA-broadcast the den row and invert all 64 rows
                        den_sb = npool.tile([D + 1, 512], F32, tag="den")
                        if i == 0:
                            nc.scalar.copy(
                                out=den_sb[D : D + 1, :], in_=o_t[D : D + 1, :]
                            )
                        else:
                            nc.vector.tensor_copy(
                                den_sb[D : D + 1, :], o_t[D : D + 1, :]
                            )
                        den_row = den_sb[D : D + 1, :]
                        bcast = npool.tile([D, 512], F32, tag="bcast")
                        # replicate the den row 64x: step-0 on a free dim
                        # (partition dims need nonzero step), written
                        # partition-major into bcast[64, 512]
                        den_b = bass.AP(
                            tensor=den_row.tensor,
                            offset=den_row.offset,
                            ap=[list(den_row.ap[0]), [0, D]]
                            + [list(x) for x in den_row.ap[1:]],
                        )
                        nc.sync.dma_start(out=bcast, in_=den_b)
                        rec = npool.tile([D, 512], F32, tag="rec")
                        nc.vector.reciprocal_approx_fast(out=rec, in_=bcast)
                        if rbox is not None:
                            rbox.append(rec)
                        box.append(rec)
                return emit

            def make_mul(p, qb, o_a, o_b, box):
                def emit():
                    cols = slice(qb * 512, (qb + 1) * 512)
                    for hh, (o_t, bcast) in enumerate(((o_a, box[0]), (o_b, box[1]))):
                        xh = xh_sb[2 * p + hh]
                        nc.vector.tensor_mul(
                            out=xh[:, cols], in0=o_t[0:D, :], in1=bcast
                        )
                        # stitch into the pair-stacked fc input (partition
                        # shifts need a DMA; DVE lanes cannot cross
                        # partitions)
                        nc.sync.dma_start(
                            out=xt_sb[p][hh * D : (hh + 1) * D, cols],
                            in_=xh[:, cols],
                        )
                return emit

            def make_fc(q128, f, ci):
                def emit():
                    y_t = opool.tile([128, 512], F32, tag="o", name="y_t")
                    for p in range(NP):
                        nc.tensor.matmul(
                            y_t,
                            lhsT=(xt_sb[p][:, q128 * 128 : (q128 + 1) * 128]),
                            rhs=(wt_sb[p][:, f * 512 : (f + 1) * 512]),
                            start=(p == 0),
                            stop=(p == NP - 1),
                        )
                    y_sb = ysb_pool.tile([128, 512], BF16, tag="ysb")
                    if ci % 2 == 0:
                        nc.scalar.copy(out=y_sb, in_=y_t)
                    else:
                        nc.vector.tensor_copy(y_sb, y_t)
                    nc.sync.dma_start(
                        out=yp[
                            q128 * 128 : (q128 + 1) * 128, f * 512 : (f + 1) * 512
                        ],
                        in_=y_sb,
                    )
                return emit

            for qb in range(QB):
                for p in range(NP):
                    qs_lo = qt_sb[p][0:D, qb * 512 : (qb + 1) * 512]
                    qs_hi = qt_sb[p][D : 2 * D, qb * 512 : (qb + 1) * 512]
                    o_a = opool.tile([D + 1, 512], F32, tag="o", name="o_a")
                    o_b = opool.tile([D + 1, 512], F32, tag="o", name="o_b")
                    for g in range(KC):
                        e_t = epool.tile([128, 1024], F32)
                        if pair_qk:
                            nc.tensor.matmul(
                                e_t[:, 0:512],
                                lhsT=(kt_sb[p][0:D, g * 128 : (g + 1) * 128]),
                                rhs=(qs_lo),
                                start=True,
                                stop=True,
                            )
                            nc.tensor.matmul(
                                e_t[:, 512:1024],
                                lhsT=(kt_sb[p][D : 2 * D, g * 128 : (g + 1) * 128]),
                                rhs=(qs_hi),
                                start=True,
                                stop=True,
                            )
                        else:
                            for hh in range(2):
                                nc.tensor.matmul(
                                    e_t[:, hh * 512 : (hh + 1) * 512],
                                    lhsT=(
                                        kh_sb[2 * p + hh][
                                            :, g * 128 : (g + 1) * 128
                                        ]
                                    ),
                                    rhs=(
                                        qh_sb[2 * p + hh][
                                            :, qb * 512 : (qb + 1) * 512
                                        ]
                                    ),
                                    start=True,
                                    stop=True,
                                )
                        a_t = apool.tile([128, 1024], BF16)
                        if exp_idx % 16 in dve_slots:
                            nc.vector._custom_dve(
                                exp_op, out=a_t, in0=e_t, s0=B2, s1=B1, imm2=B0
                            )
                        else:
                            nc.scalar.activation(
                                out=a_t,
                                in_=e_t,
                                func=mybir.ActivationFunctionType.Exp,
                                scale=ACT_SCALE,
                            )
                        exp_idx += 1
                        if debug_x and qb == 0 and p == 0 and g == 0:
                            e_dump = singles.tile([128, 1024], F32, tag="edump")
                            nc.vector.tensor_copy(e_dump, e_t)
                            nc.sync.dma_start(out=edbg[:, :], in_=e_dump)
                            nc.sync.dma_start(out=adbg[:, :], in_=a_t)
                        if pend_att is not None:
                            pend_att()
                        pend_att = make_att(p, a_t, o_a, o_b, g)
                        # deferred normalization from the previous superstep
                        if g == 4 and pend_recip is not None:
                            pend_recip()
                            pend_recip = None
                        if g == 8 and pend_mul is not None:
                            pend_mul()
                            pend_mul = None
                    box = []
                    rbox = []
                    pend_recip = make_recip(o_a, o_b, box, rbox)
                    pend_mul = make_mul(p, qb, o_a, o_b, box)
                    dbg_o = o_a
                    dbg_box = box
                    dbg_rbox = rbox
            # flush tail
            if pend_att is not None:
                pend_att()
                pend_att = None
            if pend_recip is not None:
                pend_recip()
                pend_recip = None
            if pend_mul is not None:
                pend_mul()
                pend_mul = None
            if debug_x:
                o_dump = singles.tile([D + 1, 512], F32, tag="odump")
                nc.vector.tensor_copy(o_dump, dbg_o)
                nc.sync.dma_start(out=odbg[:, :], in_=o_dump)
                nc.sync.dma_start(out=bdbg[:, :], in_=dbg_box[0])
                nc.sync.dma_start(out=rdbg[:, :], in_=dbg_rbox[0][0:1, :])
                for p in range(NP):
                    nc.sync.dma_start(
                        out=xdbg[p * 128 : (p + 1) * 128, :], in_=xt_sb[p]
                    )
            # fc phase: y[q, f] partial, K=128 per pair-stacked xt
            fc_ci = 0
            for q128 in range(S // 128):
                for f in range(E // 512):
                    make_fc(q128, f, fc_ci)()
                    fc_ci += 1
    nc.compile()
    return nc


class SpmdRunner:
    """Build one jitted shard_map callable over 8 cores; reusable for timing."""

    def __init__(self, nc, n_cores):
        import jax
        from jax.experimental.shard_map import shard_map
        from jax.sharding import Mesh, PartitionSpec

        from concourse import mybir
        from concourse.bass2jax import _bass_exec_p, install_neuronx_cc_hook
        from concourse.bass2jax import partition_id_tensor as _pid

        install_neuronx_cc_hook()
        self.jax = jax
        self.nc = nc
        self.n_cores = n_cores
        self.PartitionSpec = PartitionSpec

        partition_name = nc.partition_id_tensor.name if nc.partition_id_tensor else None
        in_names, out_names, out_avals = [], [], []
        for alloc in nc.m.functions[0].allocations:
            if not isinstance(alloc, mybir.MemoryLocationSet):
                continue
            name = alloc.memorylocations[0].name
            if alloc.kind == "ExternalInput":
                if name != partition_name:
                    in_names.append(name)
            elif alloc.kind == "ExternalOutput":
                out_names.append(name)
                shape = tuple(alloc.tensor_shape)
                dtype = mybir.dt.np(alloc.dtype)
                out_avals.append(jax.core.ShapedArray(shape, dtype))
        self.in_names = in_names
        self.out_names = out_names
        self.out_avals = out_avals
        n_params = len(in_names)
        n_outs = len(out_avals)

        all_in_names = list(in_names) + list(out_names)
        if partition_name is not None:
            all_in_names.append(partition_name)

        def _body(*args):
            operands = list(args)
            if partition_name is not None:
                operands.append(_pid())
            outs = _bass_exec_p.bind(
                *operands,
                out_avals=tuple(out_avals),
                in_names=tuple(all_in_names),
                out_names=tuple(out_names),
                lowering_input_output_aliases=(),
                sim_require_finite=True,
                sim_require_nnan=True,
                nc=nc,
            )
            return tuple(outs)

        self._body = _body
        devices = jax.devices()[:n_cores]
        assert len(devices) == n_cores
        self.mesh = Mesh(np.asarray(devices), ("core",))
        in_specs = (PartitionSpec("core"),) * (n_params + n_outs)
        out_specs = (PartitionSpec("core"),) * n_outs
        self.fn = jax.jit(
            shard_map(
                _body,
                mesh=self.mesh,
                in_specs=in_specs,
                out_specs=out_specs,
                check_rep=False,
            ),
            keep_unused=True,
        )
        self._chain_fns = {}

    def prepare(self, in_maps):
        jax = self.jax
        n = self.n_cores
        concat_in = [
            np.concatenate([np.asarray(in_maps[c][name]) for c in range(n)], axis=0)
            for name in self.in_names
        ]
        concat_zeros = [
            np.zeros((n * a.shape[0], *a.shape[1:]), a.dtype) for a in self.out_avals
        ]
        sharding = jax.sharding.NamedSharding(self.mesh, self.PartitionSpec("core"))
        self.dev_args = [jax.device_put(a, sharding) for a in concat_in + concat_zeros]
        return self.dev_args

    def run(self):
        outs = self.fn(*self.dev_args)
        self.jax.block_until_ready(outs)
        return outs

    def results(self, outs):
        n = self.n_cores
        res = []
        for c in range(n):
            d = {}
            for i, name in enumerate(self.out_names):
                a = np.asarray(outs[i])
                d[name] = a.reshape(n, *self.out_avals[i].shape)[c]
            res.append(d)
        return res

    # ---- timing support: chain K invocations through the tok tensor ----
    def chain_fn(self, k):
        if k in self._chain_fns:
            return self._chain_fns[k]
        jax = self.jax
        from jax.experimental.shard_map import shard_map

        tok_in_idx = self.in_names.index("tok")
        tok_out_idx = self.out_names.index("tok_out")
        n_params = len(self.in_names)

        def _chained(*args):
            args = list(args)
            outs = None
            for _ in range(k):
                outs = self._body(*args)
                args[tok_in_idx] = outs[tok_out_idx]
            return tuple(outs)

        in_specs = (self.PartitionSpec("core"),) * (n_params + len(self.out_names))
        out_specs = (self.PartitionSpec("core"),) * len(self.out_names)
        fn = jax.jit(
            shard_map(
                _chained,
                mesh=self.mesh,
                in_specs=in_specs,
                out_specs=out_specs,
                check_rep=False,
            ),
            keep_unused=True,
        )
        self._chain_fns[k] = fn
        return fn

    def time_chain(self, k, iters=8, warmup=2):
        fn = self.chain_fn(k)
        for _ in range(warmup):
            self.jax.block_until_ready(fn(*self.dev_args))
        ts = []
        for _ in range(iters):
            t0 = time.perf_counter()
            self.jax.block_until_ready(fn(*self.dev_args))
            ts.append(time.perf_counter() - t0)
        return min(ts)


def shard_inputs(values, keys, query, W_out):
    """Build the 8 per-core input maps (host-side layout prep)."""
    import ml_dtypes

    BF = np.dtype(ml_dtypes.bfloat16)
    v4 = np.asarray(values, np.float32).reshape(NB, S, H, D)
    k4 = np.asarray(keys, np.float32).reshape(NB, S, H, D)
    q4 = np.asarray(query, np.float32).reshape(NB, S, H, D)
    W_out = np.asarray(W_out, np.float32)
    in_maps = []
    tok = np.zeros((1, 128), np.float32)
    for c in range(N_CORES):
        n = c // 4
        h0 = HPC * (c % 4)
        # [NP, 128, S]: rows 0-63 head (h0+2p), rows 64-127 head (h0+2p+1)
        qt = np.ascontiguousarray(
            (q4[n, :, h0 : h0 + HPC, :].transpose(1, 2, 0) * np.float32(S_Q))
            .reshape(NP, 128, S)
            .astype(BF)
        )
        kt = np.ascontiguousarray(
            k4[n, :, h0 : h0 + HPC, :].transpose(1, 2, 0).reshape(NP, 128, S)
            .astype(BF)
        )
        vb = np.concatenate(
            [
                np.ascontiguousarray(v4[n, :, h0 : h0 + HPC, :].transpose(1, 0, 2)),
                np.ones((HPC, S, 1), np.float32),
            ],
            axis=2,
        ).astype(BF)  # [HPC, S, D+1]
        wt = np.ascontiguousarray(
            W_out[:, (h0 * D) : (h0 + HPC) * D].T.reshape(NP, 128, E).astype(BF)
        )
        in_maps.append({"qt": qt, "kt": kt, "vb": vb, "wt": wt, "tok": tok})
    return in_maps


_CACHE = {}


def get_runner():
    if "runner" not in _CACHE:
        nc = build_kernel()
        _CACHE["runner"] = SpmdRunner(nc, N_CORES)
    return _CACHE["runner"]


def kernel(values, keys, query, W_out, b_out):
    runner = get_runner()
    in_maps = shard_inputs(values, keys, query, W_out)
    runner.prepare(in_maps)
    outs = runner.run()
    res = runner.results(outs)
    y = np.zeros((NB, S, E), np.float32)
    for c in range(N_CORES):
        y[c // 4] += res[c]["yp"].astype(np.float32)
    y += np.asarray(b_out, np.float32)[None, None, :]
    return y


# revision 30
# speedup vs baseline: 3.9149x; 1.2458x over previous
"""Trainium2 Bass kernel for nn_MultiHeadSelfAttention (N=2, S=2048, E=1024, H=16).

Sharding: heads+batch tensor-parallel over 8 cores. Core c handles batch
n = c // 4 and 4 heads h in [4*(c%4), 4*(c%4)+4), organized as 2 pairs.
Attention is computed per-head in a transposed layout (contraction dims
on SBUF partitions); fc_out is row-parallel: each core computes a partial
y over its 256 embedding dims, and the host sums 4 partials per batch and
adds bias.

v2 (vs fp32r baseline): all PE operands bf16 (fp32 PSUM accumulation),
QK^T row-tiled in head pairs (two K=64 matmuls run concurrently in the
PE array via base-partition-derived tile_position), fc contracts over
pair-stacked K=128, normalization reciprocal reads the denominator row
straight from PSUM.

Per-core pipeline per (pair, qblock):
  for g in 16 key chunks:
    e[:, 0:512]   = KT_a[g].T @ QT_a    (rows 0-63 of the PE array)
    e[:, 512:1024]= KT_b[g].T @ QT_b    (rows 64-127, concurrent)
    a = exp(e / 32)                     (ScalarE Exp / VectorE poly8, bf16)
    oA += [V_a[g] | ones].T @ a[:, 0:512]    (ones col -> den in row 64)
    oB += [V_b[g] | ones].T @ a[:, 512:1024]
  rec = 1/den (DVE, from PSUM row 64); DMA-broadcast; x = o * rec -> xt
fc: y[q,f] = sum_p xt_p[:, q].T @ W_p  (K=128 per pair, 2-mm accumulate)
"""

import time

import numpy as np

N_CORES = 8
NB = 2          # batch
S = 2048        # sequence length
E = 1024        # embed size
H = 16          # heads
D = 64          # head dim
HPC = 4         # heads per core
NP = 2          # head pairs per core
SCALE = float(1.0 / np.sqrt(E))  # softmax scale (embed_size**0.5)

KC = S // 128   # 16 contraction chunks of 128 keys
QB = S // 512   # 4 query blocks of 512

# exp(x) ~= p(t)^8, p monic cubic, t = x pre-scaled by S_Q on the host.
# ScalarE chunks use exp(t * ACT_SCALE) (exact); VectorE chunks use the
# 8-stage custom DVE polynomial (rel err ~2.6e-4).
ALPHA = 6.0 ** (1.0 / 3.0)
S_Q = float(SCALE / (8.0 * ALPHA))   # host pre-scale on Q (t = S_Q/SCALE * x)
ACT_SCALE = float(8.0 * ALPHA)
B2, B1, B0 = 1.6574587989430332, 1.8171403999384372, 0.9999891634709047

# exp engine split: DVE takes these slots of each 16-slot cycle (7/16)
DVE_SLOTS = frozenset((1, 3, 5, 7, 9, 11, 13))


def register_exp_op():
    """Register the EXP_POLY8_ANT custom DVE op (idempotent)."""
    import concourse.dve_ops as dve_ops
    from concourse.dve_ops import OPS, DveOp
    from concourse.dve_spec import C0, C1, C2, Spec, Src0, _has_src1, lower, sq
    from concourse.dve_uop import DveOpSpec

    name = "EXP_POLY8_ANT"
    for op in OPS:
        if op.name == name:
            return op

    _p = ((Src0 + C0) * Src0 + C1) * Src0 + C2
    _body = sq(sq(sq(_p)))

    def _ref(in0, in1, s0, s1, imm2):
        p = ((in0 + s0) * in0 + s1) * in0 + imm2
        return ((p ** 2) ** 2) ** 2

    spec = Spec(body=_body, reference=_ref)
    opcode = dve_ops._CUSTOM_DVE_ROW_BASE + len(OPS)
    shas = {}
    for ver in ("v3", "v4"):
        d = DveOpSpec(
            name=name, opcode=opcode, uops=lower(spec, ver=ver),
            rd1_en=_has_src1(spec),
        )
        shas[ver] = d.sha(ver)
    op = DveOp(name, spec, subdim=False, uops_sha=shas)
    OPS.append(op)
    dve_ops._SUB_OPCODE_FOR_NAME[name] = opcode
    dve_ops.CUSTOM_DVE_SPECS[name] = spec
    return op


def build_kernel(
    reps=1, dve_slots=DVE_SLOTS, abufs=4, ybufs=4, pair_qk=True, debug_x=False
):
    import contextlib

    import concourse.bacc as bacc
    import concourse.bass as bass
    import concourse.tile as tile
    from concourse import mybir

    F32 = mybir.dt.float32
    BF16 = mybir.dt.bfloat16

    exp_op = register_exp_op()

    nc = bacc.Bacc("TRN2", target_bir_lowering=False, num_devices=N_CORES)

    # pair-stacked inputs: rows 0-63 head 2p, rows 64-127 head 2p+1
    qt = nc.dram_tensor("qt", [NP, 128, S], BF16, kind="ExternalInput")
    kt = nc.dram_tensor("kt", [NP, 128, S], BF16, kind="ExternalInput")
    vb = nc.dram_tensor("vb", [HPC, S, D + 1], BF16, kind="ExternalInput")
    wt = nc.dram_tensor("wt", [NP, 128, E], BF16, kind="ExternalInput")
    tok = nc.dram_tensor("tok", [1, 128], F32, kind="ExternalInput")
    yp = nc.dram_tensor("yp", [S, E], BF16, kind="ExternalOutput")
    tok_out = nc.dram_tensor("tok_out", [1, 128], F32, kind="ExternalOutput")
    if debug_x:
        xdbg = nc.dram_tensor("xdbg", [NP * 128, S], BF16, kind="ExternalOutput")
        edbg = nc.dram_tensor("edbg", [128, 1024], F32, kind="ExternalOutput")
        adbg = nc.dram_tensor("adbg", [128, 1024], BF16, kind="ExternalOutput")
        odbg = nc.dram_tensor("odbg", [D + 1, 512], F32, kind="ExternalOutput")
        bdbg = nc.dram_tensor("bdbg", [D, 512], F32, kind="ExternalOutput")
        rdbg = nc.dram_tensor("rdbg", [1, 512], F32, kind="ExternalOutput")

    with tile.TileContext(nc) as tc:
        with contextlib.ExitStack() as ctx:
            singles = ctx.enter_context(tc.tile_pool(name="singles", bufs=1))
            epool = ctx.enter_context(
                tc.tile_pool(name="epool", bufs=2, space="PSUM")
            )
            opool = ctx.enter_context(
                tc.tile_pool(name="opool", bufs=4, space="PSUM")
            )
            apool = ctx.enter_context(tc.tile_pool(name="apool", bufs=abufs))
            npool = ctx.enter_context(tc.tile_pool(name="npool", bufs=4))
            ysb_pool = ctx.enter_context(tc.tile_pool(name="ysb", bufs=ybufs))

            # token passthrough for timing chains
            tok_sb = singles.tile([1, 128], F32)
            nc.gpsimd.dma_start(out=tok_sb, in_=tok[:, :])
            nc.gpsimd.dma_start(out=tok_out[:, :], in_=tok_sb)

            # resident inputs
            qt_sb, kt_sb, wt_sb, xt_sb, v_sb = [], [], [], [], []
            qh_sb, kh_sb = [], []
            for p in range(NP):
                q_t = singles.tile([128, S], BF16, tag=f"qt{p}")
                nc.sync.dma_start(out=q_t, in_=qt[p])
                qt_sb.append(q_t)
                k_t = singles.tile([128, S], BF16, tag=f"kt{p}")
                nc.sync.dma_start(out=k_t, in_=kt[p])
                kt_sb.append(k_t)
                if not pair_qk:
                    for hh in range(2):
                        q_h = singles.tile([D, S], BF16, tag=f"qh{p}_{hh}")
                        nc.sync.dma_start(
                            out=q_h, in_=qt[p][hh * D : (hh + 1) * D, :]
                        )
                        qh_sb.append(q_h)
                        k_h = singles.tile([D, S], BF16, tag=f"kh{p}_{hh}")
                        nc.sync.dma_start(
                            out=k_h, in_=kt[p][hh * D : (hh + 1) * D, :]
                        )
                        kh_sb.append(k_h)
                w_t = singles.tile([128, E], BF16, tag=f"wt{p}")
                nc.sync.dma_start(out=w_t, in_=wt[p])
                wt_sb.append(w_t)
                x_t = singles.tile([128, S], BF16, tag=f"xt{p}")
                xt_sb.append(x_t)
            xh_sb = []
            for h in range(HPC):
                xh = singles.tile([D, S], BF16, tag=f"xh{h}")
                xh_sb.append(xh)
            for h in range(HPC):
                v_t = singles.tile([128, KC, D + 1], BF16, tag=f"v{h}")
                nc.sync.dma_start(
                    out=v_t, in_=vb[h].rearrange("(kc p) c -> p kc c", p=128)
                )
                v_sb.append(v_t)

            loop_cm = tc.For_i(0, reps, 1) if reps > 1 else contextlib.nullcontext()
            ctx.enter_context(loop_cm)

            exp_idx = 0
            pend_att = None    # deferred attV emission (one group behind)
            pend_recip = None  # deferred reciprocal (into next superstep)
            pend_mul = None    # deferred normalize-multiply

            def make_att(p, a_t, o_a, o_b, g):
                def emit():
                    nc.tensor.matmul(
                        o_a,
                        lhsT=(v_sb[2 * p][:, g, :]),
                        rhs=(a_t[:, 0:512]),
                        start=(g == 0),
                        stop=(g == KC - 1),
                    )
                    nc.tensor.matmul(
                        o_b,
                        lhsT=(v_sb[2 * p + 1][:, g, :]),
                        rhs=(a_t[:, 512:1024]),
                        start=(g == 0),
                        stop=(g == KC - 1),
                    )
                return emit

            def make_recip(o_a, o_b, box, rbox=None):
                def emit():
                    for i, o_t in enumerate((o_a, o_b)):
                        # lane-aligned den copy (partition 64 -> 64), then
                        # DMA-broadcast the den row and invert all 64 rows
                        den_sb = npool.tile([D + 1, 512], F32, tag="den")
                        if i == 0:
                            nc.scalar.copy(
                                out=den_sb[D : D + 1, :], in_=o_t[D : D + 1, :]
                            )
                        else:
                            nc.vector.tensor_copy(
                                den_sb[D : D + 1, :], o_t[D : D + 1, :]
                            )
                        den_row = den_sb[D : D + 1, :]
                        bcast = npool.tile([D, 512], F32, tag="bcast")
                        # replicate the den row 64x: step-0 on a free dim
                        # (partition dims need nonzero step), written
                        # partition-major into bcast[64, 512]
                        den_b = bass.AP(
                            tensor=den_row.tensor,
                            offset=den_row.offset,
                            ap=[list(den_row.ap[0]), [0, D]]
                            + [list(x) for x in den_row.ap[1:]],
                        )
                        nc.sync.dma_start(out=bcast, in_=den_b)
                        rec = npool.tile([D, 512], F32, tag="rec")
                        nc.vector.reciprocal_approx_fast(out=rec, in_=bcast)
                        if rbox is not None:
                            rbox.append(rec)
                        box.append(rec)
                return emit

            def make_mul(p, qb, o_a, o_b, box):
                def emit():
                    cols = slice(qb * 512, (qb + 1) * 512)
                    for hh, (o_t, bcast) in enumerate(((o_a, box[0]), (o_b, box[1]))):
                        xh = xh_sb[2 * p + hh]
                        nc.vector.tensor_mul(
                            out=xh[:, cols], in0=o_t[0:D, :], in1=bcast
                        )
                        # stitch into the pair-stacked fc input (partition
                        # shifts need a DMA; DVE lanes cannot cross
                        # partitions)
                        nc.sync.dma_start(
                            out=xt_sb[p][hh * D : (hh + 1) * D, cols],
                            in_=xh[:, cols],
                        )
                return emit

            def make_fc(q128, f, ci):
                def emit():
                    y_t = opool.tile([128, 512], F32, tag="o", name="y_t")
                    for p in range(NP):
                        nc.tensor.matmul(
                            y_t,
                            lhsT=(xt_sb[p][:, q128 * 128 : (q128 + 1) * 128]),
                            rhs=(wt_sb[p][:, f * 512 : (f + 1) * 512]),
                            start=(p == 0),
                            stop=(p == NP - 1),
                        )
                    y_sb = ysb_pool.tile([128, 512], BF16, tag="ysb")
                    if ci % 2 == 0:
                        nc.scalar.copy(out=y_sb, in_=y_t)
                    else:
                        nc.vector.tensor_copy(y_sb, y_t)
                    nc.sync.dma_start(
                        out=yp[
                            q128 * 128 : (q128 + 1) * 128, f * 512 : (f + 1) * 512
                        ],
                        in_=y_sb,
                    )
                return emit

            for qb in range(QB):
                for p in range(NP):
                    qs_lo = qt_sb[p][0:D, qb * 512 : (qb + 1) * 512]
                    qs_hi = qt_sb[p][D : 2 * D, qb * 512 : (qb + 1) * 512]
                    o_a = opool.tile([D + 1, 512], F32, tag="o", name="o_a")
                    o_b = opool.tile([D + 1, 512], F32, tag="o", name="o_b")
                    for g in range(KC):
                        e_t = epool.tile([128, 1024], F32)
                        if pair_qk:
                            nc.tensor.matmul(
                                e_t[:, 0:512],
                                lhsT=(kt_sb[p][0:D, g * 128 : (g + 1) * 128]),
                                rhs=(qs_lo),
                                start=True,
                                stop=True,
                            )
                            nc.tensor.matmul(
                                e_t[:, 512:1024],
                                lhsT=(kt_sb[p][D : 2 * D, g * 128 : (g + 1) * 128]),
                                rhs=(qs_hi),
                                start=True,
                                stop=True,
                            )
                        else:
                            for hh in range(2):
                                nc.tensor.matmul(
                                    e_t[:, hh * 512 : (hh + 1) * 512],
                                    lhsT=(
                                        kh_sb[2 * p + hh][
                                            :, g * 128 : (g + 1) * 128
                                        ]
                                    ),
                                    rhs=(
                                        qh_sb[2 * p + hh][
                                            :, qb * 512 : (qb + 1) * 512
                                        ]
                                    ),
                                    start=True,
                                    stop=True,
                                )
                        a_t = apool.tile([128, 1024], BF16)
                        if exp_idx % 16 in dve_slots:
                            nc.vector._custom_dve(
                                exp_op, out=a_t, in0=e_t, s0=B2, s1=B1, imm2=B0
                            )
                        else:
                            nc.scalar.activation(
                                out=a_t,
                                in_=e_t,
                                func=mybir.ActivationFunctionType.Exp,
                                scale=ACT_SCALE,
                            )
                        exp_idx += 1
                        if debug_x and qb == 0 and p == 0 and g == 0:
                            e_dump = singles.tile([128, 1024], F32, tag="edump")
                            nc.vector.tensor_copy(e_dump, e_t)
                            nc.sync.dma_start(out=edbg[:, :], in_=e_dump)
                            nc.sync.dma_start(out=adbg[:, :], in_=a_t)
                        if pend_att is not None:
                            pend_att()
                        pend_att = make_att(p, a_t, o_a, o_b, g)
                        # deferred normalization from the previous superstep
                        if g == 4 and pend_recip is not None:
                            pend_recip()
                            pend_recip = None
                        if g == 8 and pend_mul is not None:
                            pend_mul()
                            pend_mul = None
                    box = []
                    rbox = []
                    pend_recip = make_recip(o_a, o_b, box, rbox)
                    pend_mul = make_mul(p, qb, o_a, o_b, box)
                    dbg_o = o_a
                    dbg_box = box
                    dbg_rbox = rbox
            # flush tail
            if pend_att is not None:
                pend_att()
                pend_att = None
            if pend_recip is not None:
                pend_recip()
                pend_recip = None
            if pend_mul is not None:
                pend_mul()
                pend_mul = None
            if debug_x:
                o_dump = singles.tile([D + 1, 512], F32, tag="odump")
                nc.vector.tensor_copy(o_dump, dbg_o)
                nc.sync.dma_start(out=odbg[:, :], in_=o_dump)
                nc.sync.dma_start(out=bdbg[:, :], in_=dbg_box[0])
                nc.sync.dma_start(out=rdbg[:, :], in_=dbg_rbox[0][0:1, :])
                for p in range(NP):
                    nc.sync.dma_start(
                        out=xdbg[p * 128 : (p + 1) * 128, :], in_=xt_sb[p]
                    )
            # fc phase: y[q, f] partial, K=128 per pair-stacked xt
            fc_ci = 0
            for q128 in range(S // 128):
                for f in range(E // 512):
                    make_fc(q128, f, fc_ci)()
                    fc_ci += 1
    nc.compile()
    return nc


class SpmdRunner:
    """Build one jitted shard_map callable over 8 cores; reusable for timing."""

    def __init__(self, nc, n_cores):
        import jax
        from jax.experimental.shard_map import shard_map
        from jax.sharding import Mesh, PartitionSpec

        from concourse import mybir
        from concourse.bass2jax import _bass_exec_p, install_neuronx_cc_hook
        from concourse.bass2jax import partition_id_tensor as _pid

        install_neuronx_cc_hook()
        self.jax = jax
        self.nc = nc
        self.n_cores = n_cores
        self.PartitionSpec = PartitionSpec

        partition_name = nc.partition_id_tensor.name if nc.partition_id_tensor else None
        in_names, out_names, out_avals = [], [], []
        for alloc in nc.m.functions[0].allocations:
            if not isinstance(alloc, mybir.MemoryLocationSet):
                continue
            name = alloc.memorylocations[0].name
            if alloc.kind == "ExternalInput":
                if name != partition_name:
                    in_names.append(name)
            elif alloc.kind == "ExternalOutput":
                out_names.append(name)
                shape = tuple(alloc.tensor_shape)
                dtype = mybir.dt.np(alloc.dtype)
                out_avals.append(jax.core.ShapedArray(shape, dtype))
        self.in_names = in_names
        self.out_names = out_names
        self.out_avals = out_avals
        n_params = len(in_names)
        n_outs = len(out_avals)

        all_in_names = list(in_names) + list(out_names)
        if partition_name is not None:
            all_in_names.append(partition_name)

        def _body(*args):
            operands = list(args)
            if partition_name is not None:
                operands.append(_pid())
            outs = _bass_exec_p.bind(
                *operands,
                out_avals=tuple(out_avals),
                in_names=tuple(all_in_names),
                out_names=tuple(out_names),
                lowering_input_output_aliases=(),
                sim_require_finite=True,
                sim_require_nnan=True,
                nc=nc,
            )
            return tuple(outs)

        self._body = _body
        devices = jax.devices()[:n_cores]
        assert len(devices) == n_cores
        self.mesh = Mesh(np.asarray(devices), ("core",))
        in_specs = (PartitionSpec("core"),) * (n_params + n_outs)
        out_specs = (PartitionSpec("core"),) * n_outs
        self.fn = jax.jit(
            shard_map(
                _body,
                mesh=self.mesh,
                in_specs=in_specs,
                out_specs=out_specs,
                check_rep=False,
            ),
            keep_unused=True,
        )
        self._chain_fns = {}

    def prepare(self, in_maps):
        jax = self.jax
        n = self.n_cores
        concat_in = [
            np.concatenate([np.asarray(in_maps[c][name]) for c in range(n)], axis=0)
            for name in self.in_names
        ]
        concat_zeros = [
            np.zeros((n * a.shape[0], *a.shape[1:]), a.dtype) for a in self.out_avals
        ]
        sharding = jax.sharding.NamedSharding(self.mesh, self.PartitionSpec("core"))
        self.dev_args = [jax.device_put(a, sharding) for a in concat_in + concat_zeros]
        return self.dev_args

    def run(self):
        outs = self.fn(*self.dev_args)
        self.jax.block_until_ready(outs)
        return outs

    def results(self, outs):
        n = self.n_cores
        res = []
        for c in range(n):
            d = {}
            for i, name in enumerate(self.out_names):
                a = np.asarray(outs[i])
                d[name] = a.reshape(n, *self.out_avals[i].shape)[c]
            res.append(d)
        return res

    # ---- timing support: chain K invocations through the tok tensor ----
    def chain_fn(self, k):
        if k in self._chain_fns:
            return self._chain_fns[k]
        jax = self.jax
        from jax.experimental.shard_map import shard_map

        tok_in_idx = self.in_names.index("tok")
        tok_out_idx = self.out_names.index("tok_out")
        n_params = len(self.in_names)

        def _chained(*args):
            args = list(args)
            outs = None
            for _ in range(k):
                outs = self._body(*args)
                args[tok_in_idx] = outs[tok_out_idx]
            return tuple(outs)

        in_specs = (self.PartitionSpec("core"),) * (n_params + len(self.out_names))
        out_specs = (self.PartitionSpec("core"),) * len(self.out_names)
        fn = jax.jit(
            shard_map(
                _chained,
                mesh=self.mesh,
                in_specs=in_specs,
                out_specs=out_specs,
                check_rep=False,
            ),
            keep_unused=True,
        )
        self._chain_fns[k] = fn
        return fn

    def time_chain(self, k, iters=8, warmup=2):
        fn = self.chain_fn(k)
        for _ in range(warmup):
            self.jax.block_until_ready(fn(*self.dev_args))
        ts = []
        for _ in range(iters):
            t0 = time.perf_counter()
            self.jax.block_until_ready(fn(*self.dev_args))
            ts.append(time.perf_counter() - t0)
        return min(ts)


def shard_inputs(values, keys, query, W_out):
    """Build the 8 per-core input maps (host-side layout prep)."""
    import ml_dtypes

    BF = np.dtype(ml_dtypes.bfloat16)
    v4 = np.asarray(values, np.float32).reshape(NB, S, H, D)
    k4 = np.asarray(keys, np.float32).reshape(NB, S, H, D)
    q4 = np.asarray(query, np.float32).reshape(NB, S, H, D)
    W_out = np.asarray(W_out, np.float32)
    in_maps = []
    tok = np.zeros((1, 128), np.float32)
    for c in range(N_CORES):
        n = c // 4
        h0 = HPC * (c % 4)
        # [NP, 128, S]: rows 0-63 head (h0+2p), rows 64-127 head (h0+2p+1)
        qt = np.ascontiguousarray(
            (q4[n, :, h0 : h0 + HPC, :].transpose(1, 2, 0) * np.float32(S_Q))
            .reshape(NP, 128, S)
            .astype(BF)
        )
        kt = np.ascontiguousarray(
            k4[n, :, h0 : h0 + HPC, :].transpose(1, 2, 0).reshape(NP, 128, S)
            .astype(BF)
        )
        vb = np.concatenate(
            [
                np.ascontiguousarray(v4[n, :, h0 : h0 + HPC, :].transpose(1, 0, 2)),
                np.ones((HPC, S, 1), np.float32),
            ],
            axis=2,
        ).astype(BF)  # [HPC, S, D+1]
        wt = np.ascontiguousarray(
            W_out[:, (h0 * D) : (h0 + HPC) * D].T.reshape(NP, 128, E).astype(BF)
        )
        in_maps.append({"qt": qt, "kt": kt, "vb": vb, "wt": wt, "tok": tok})
    return in_maps


_CACHE = {}


def get_runner():
    if "runner" not in _CACHE:
        nc = build_kernel()
        _CACHE["runner"] = SpmdRunner(nc, N_CORES)
    return _CACHE["runner"]


def kernel(values, keys, query, W_out, b_out):
    runner = get_runner()
    in_maps = shard_inputs(values, keys, query, W_out)
    runner.prepare(in_maps)
    outs = runner.run()
    res = runner.results(outs)
    y = np.zeros((NB, S, E), np.float32)
    for c in range(N_CORES):
        y[c // 4] += res[c]["yp"].astype(np.float32)
    y += np.asarray(b_out, np.float32)[None, None, :]
    return y


# revision 34
# speedup vs baseline: 5.4822x; 1.4004x over previous
"""Trainium2 Bass kernel for nn_MultiHeadSelfAttention (N=2, S=2048, E=1024, H=16).

Sharding: heads+batch tensor-parallel over 8 cores. Core c handles batch
n = c // 4 and 4 heads h in [4*(c%4), 4*(c%4)+4), organized as 2 pairs.
Attention is computed per-head in a transposed layout (contraction dims
on SBUF partitions); fc_out is row-parallel: each core computes a partial
y over its 256 embedding dims, and the host sums 4 partials per batch and
adds bias.

v2 (vs fp32r baseline): all PE operands bf16 (fp32 PSUM accumulation),
QK^T row-tiled in head pairs (two K=64 matmuls run concurrently in the
PE array via base-partition-derived tile_position), fc contracts over
pair-stacked K=128, normalization reciprocal reads the denominator row
straight from PSUM.

Per-core pipeline per (pair, qblock):
  for g in 16 key chunks:
    e[:, 0:512]   = KT_a[g].T @ QT_a    (rows 0-63 of the PE array)
    e[:, 512:1024]= KT_b[g].T @ QT_b    (rows 64-127, concurrent)
    a = exp(e / 32)                     (ScalarE Exp / VectorE poly8, bf16)
    oA += [V_a[g] | ones].T @ a[:, 0:512]    (ones col -> den in row 64)
    oB += [V_b[g] | ones].T @ a[:, 512:1024]
  rec = 1/den (DVE, from PSUM row 64); DMA-broadcast; x = o * rec -> xt
fc: y[q,f] = sum_p xt_p[:, q].T @ W_p  (K=128 per pair, 2-mm accumulate)
"""

import time

import numpy as np

N_CORES = 8
NB = 2          # batch
S = 2048        # sequence length
E = 1024        # embed size
H = 16          # heads
D = 64          # head dim
HPC = 4         # heads per core
NP = 2          # head pairs per core
SCALE = float(1.0 / np.sqrt(E))  # softmax scale (embed_size**0.5)

KC = S // 128   # 16 contraction chunks of 128 keys
QB = S // 512   # 4 query blocks of 512

# exp(x) ~= p(t)^8, p monic cubic, t = x pre-scaled by S_Q on the host.
# ScalarE chunks use exp(t * ACT_SCALE) (exact); VectorE chunks use the
# 8-stage custom DVE polynomial (rel err ~2.6e-4).
ALPHA = 6.0 ** (1.0 / 3.0)
S_Q = float(SCALE / (8.0 * ALPHA))   # host pre-scale on Q (t = S_Q/SCALE * x)
ACT_SCALE = float(8.0 * ALPHA)
B2, B1, B0 = 1.6574587989430332, 1.8171403999384372, 0.9999891634709047

# exp engine split: DVE takes these slots of each 16-slot cycle (7/16)
DVE_SLOTS = frozenset((1, 3, 5, 7, 9, 11, 13))


def register_exp_op():
    """Register the EXP_POLY8_ANT custom DVE op (idempotent)."""
    import concourse.dve_ops as dve_ops
    from concourse.dve_ops import OPS, DveOp
    from concourse.dve_spec import C0, C1, C2, Spec, Src0, _has_src1, lower, sq
    from concourse.dve_uop import DveOpSpec

    name = "EXP_POLY8_ANT"
    for op in OPS:
        if op.name == name:
            return op

    _p = ((Src0 + C0) * Src0 + C1) * Src0 + C2
    _body = sq(sq(sq(_p)))

    def _ref(in0, in1, s0, s1, imm2):
        p = ((in0 + s0) * in0 + s1) * in0 + imm2
        return ((p ** 2) ** 2) ** 2

    spec = Spec(body=_body, reference=_ref)
    opcode = dve_ops._CUSTOM_DVE_ROW_BASE + len(OPS)
    shas = {}
    for ver in ("v3", "v4"):
        d = DveOpSpec(
            name=name, opcode=opcode, uops=lower(spec, ver=ver),
            rd1_en=_has_src1(spec),
        )
        shas[ver] = d.sha(ver)
    op = DveOp(name, spec, subdim=False, uops_sha=shas)
    OPS.append(op)
    dve_ops._SUB_OPCODE_FOR_NAME[name] = opcode
    dve_ops.CUSTOM_DVE_SPECS[name] = spec
    return op


def build_kernel(
    reps=1, dve_slots=DVE_SLOTS, abufs=4, ybufs=4, pair_qk=True, debug_x=False
):
    import contextlib

    import concourse.bacc as bacc
    import concourse.bass as bass
    import concourse.tile as tile
    from concourse import mybir

    F32 = mybir.dt.float32
    BF16 = mybir.dt.bfloat16

    exp_op = register_exp_op()

    nc = bacc.Bacc("TRN2", target_bir_lowering=False, num_devices=N_CORES)

    # pair-stacked inputs: rows 0-63 head 2p, rows 64-127 head 2p+1
    qt = nc.dram_tensor("qt", [NP, 128, S], BF16, kind="ExternalInput")
    kt = nc.dram_tensor("kt", [NP, 128, S], BF16, kind="ExternalInput")
    vb = nc.dram_tensor("vb", [HPC, S, D + 1], BF16, kind="ExternalInput")
    wt = nc.dram_tensor("wt", [NP, 128, E], BF16, kind="ExternalInput")
    tok = nc.dram_tensor("tok", [1, 128], F32, kind="ExternalInput")
    yp = nc.dram_tensor("yp", [S, E], BF16, kind="ExternalOutput")
    tok_out = nc.dram_tensor("tok_out", [1, 128], F32, kind="ExternalOutput")
    if debug_x:
        xdbg = nc.dram_tensor("xdbg", [NP * 128, S], BF16, kind="ExternalOutput")
        edbg = nc.dram_tensor("edbg", [128, 1024], F32, kind="ExternalOutput")
        adbg = nc.dram_tensor("adbg", [128, 1024], BF16, kind="ExternalOutput")
        odbg = nc.dram_tensor("odbg", [D + 1, 512], F32, kind="ExternalOutput")
        bdbg = nc.dram_tensor("bdbg", [D, 512], F32, kind="ExternalOutput")
        rdbg = nc.dram_tensor("rdbg", [1, 512], F32, kind="ExternalOutput")

    with tile.TileContext(nc) as tc:
        with contextlib.ExitStack() as ctx:
            singles = ctx.enter_context(tc.tile_pool(name="singles", bufs=1))
            epool = ctx.enter_context(
                tc.tile_pool(name="epool", bufs=2, space="PSUM")
            )
            opool = ctx.enter_context(
                tc.tile_pool(name="opool", bufs=4, space="PSUM")
            )
            apool = ctx.enter_context(tc.tile_pool(name="apool", bufs=abufs))
            npool = ctx.enter_context(tc.tile_pool(name="npool", bufs=4))
            ysb_pool = ctx.enter_context(tc.tile_pool(name="ysb", bufs=ybufs))

            # token passthrough for timing chains
            tok_sb = singles.tile([1, 128], F32)
            nc.gpsimd.dma_start(out=tok_sb, in_=tok[:, :])
            nc.gpsimd.dma_start(out=tok_out[:, :], in_=tok_sb)

            # resident inputs
            qt_sb, kt_sb, wt_sb, xt_sb, v_sb = [], [], [], [], []
            qh_sb, kh_sb = [], []
            for p in range(NP):
                q_t = singles.tile([128, S], BF16, tag=f"qt{p}")
                nc.sync.dma_start(out=q_t, in_=qt[p])
                qt_sb.append(q_t)
                k_t = singles.tile([128, S], BF16, tag=f"kt{p}")
                nc.sync.dma_start(out=k_t, in_=kt[p])
                kt_sb.append(k_t)
                if not pair_qk:
                    for hh in range(2):
                        q_h = singles.tile([D, S], BF16, tag=f"qh{p}_{hh}")
                        nc.sync.dma_start(
                            out=q_h, in_=qt[p][hh * D : (hh + 1) * D, :]
                        )
                        qh_sb.append(q_h)
                        k_h = singles.tile([D, S], BF16, tag=f"kh{p}_{hh}")
                        nc.sync.dma_start(
                            out=k_h, in_=kt[p][hh * D : (hh + 1) * D, :]
                        )
                        kh_sb.append(k_h)
                w_t = singles.tile([128, E], BF16, tag=f"wt{p}")
                nc.sync.dma_start(out=w_t, in_=wt[p])
                wt_sb.append(w_t)
                x_t = singles.tile([128, S], BF16, tag=f"xt{p}")
                xt_sb.append(x_t)
            xh_sb = []
            for h in range(HPC):
                xh = singles.tile([D, S], BF16, tag=f"xh{h}")
                xh_sb.append(xh)
            for h in range(HPC):
                v_t = singles.tile([128, KC, D + 1], BF16, tag=f"v{h}")
                nc.sync.dma_start(
                    out=v_t, in_=vb[h].rearrange("(kc p) c -> p kc c", p=128)
                )
                v_sb.append(v_t)

            loop_cm = tc.For_i(0, reps, 1) if reps > 1 else contextlib.nullcontext()
            ctx.enter_context(loop_cm)

            exp_idx = 0
            pend_att = None    # deferred attV emission (one group behind)
            pend_recip = None  # deferred reciprocal (into next superstep)
            pend_mul = None    # deferred normalize-multiply

            def make_att(p, a_t, o_a, o_b, g):
                def emit():
                    nc.tensor.matmul(
                        o_a,
                        lhsT=(v_sb[2 * p][:, g, :]),
                        rhs=(a_t[:, 0:512]),
                        start=(g == 0),
                        stop=(g == KC - 1),
                    )
                    nc.tensor.matmul(
                        o_b,
                        lhsT=(v_sb[2 * p + 1][:, g, :]),
                        rhs=(a_t[:, 512:1024]),
                        start=(g == 0),
                        stop=(g == KC - 1),
                    )
                return emit

            def make_recip(o_a, o_b, box, rbox=None):
                def emit():
                    for i, o_t in enumerate((o_a, o_b)):
                        # lane-aligned den copy (partition 64 -> 64), then
                        # DMA-broadcast the den row and invert all 64 rows
                        den_sb = npool.tile([D + 1, 512], F32, tag="den")
                        if i == 0:
                            nc.scalar.copy(
                                out=den_sb[D : D + 1, :], in_=o_t[D : D + 1, :]
                            )
                        else:
                            nc.vector.tensor_copy(
                                den_sb[D : D + 1, :], o_t[D : D + 1, :]
                            )
                        den_row = den_sb[D : D + 1, :]
                        bcast = npool.tile([D, 512], F32, tag="bcast")
                        # replicate the den row 64x: step-0 on a free dim
                        # (partition dims need nonzero step), written
                        # partition-major into bcast[64, 512]
                        den_b = bass.AP(
                            tensor=den_row.tensor,
                            offset=den_row.offset,
                            ap=[list(den_row.ap[0]), [0, D]]
                            + [list(x) for x in den_row.ap[1:]],
                        )
                        nc.sync.dma_start(out=bcast, in_=den_b)
                        rec = npool.tile([D, 512], F32, tag="rec")
                        nc.vector.reciprocal_approx_fast(out=rec, in_=bcast)
                        if rbox is not None:
                            rbox.append(rec)
                        box.append(rec)
                return emit

            def make_mul(p, qb, o_a, o_b, box):
                def emit():
                    cols = slice(qb * 512, (qb + 1) * 512)
                    for hh, (o_t, bcast) in enumerate(((o_a, box[0]), (o_b, box[1]))):
                        xh = xh_sb[2 * p + hh]
                        nc.vector.tensor_mul(
                            out=xh[:, cols], in0=o_t[0:D, :], in1=bcast
                        )
                        # stitch into the pair-stacked fc input (partition
                        # shifts need a DMA; DVE lanes cannot cross
                        # partitions)
                        nc.sync.dma_start(
                            out=xt_sb[p][hh * D : (hh + 1) * D, cols],
                            in_=xh[:, cols],
                        )
                return emit

            def make_fc(q128, f, ci):
                def emit():
                    y_t = opool.tile([128, 512], F32, tag="o", name="y_t")
                    for p in range(NP):
                        nc.tensor.matmul(
                            y_t,
                            lhsT=(xt_sb[p][:, q128 * 128 : (q128 + 1) * 128]),
                            rhs=(wt_sb[p][:, f * 512 : (f + 1) * 512]),
                            start=(p == 0),
                            stop=(p == NP - 1),
                        )
                    y_sb = ysb_pool.tile([128, 512], BF16, tag="ysb")
                    if ci % 2 == 0:
                        nc.scalar.copy(out=y_sb, in_=y_t)
                    else:
                        nc.vector.tensor_copy(y_sb, y_t)
                    nc.sync.dma_start(
                        out=yp[
                            q128 * 128 : (q128 + 1) * 128, f * 512 : (f + 1) * 512
                        ],
                        in_=y_sb,
                    )
                return emit

            for qb in range(QB):
                for p in range(NP):
                    qs_lo = qt_sb[p][0:D, qb * 512 : (qb + 1) * 512]
                    qs_hi = qt_sb[p][D : 2 * D, qb * 512 : (qb + 1) * 512]
                    o_a = opool.tile([D + 1, 512], F32, tag="o", name="o_a")
                    o_b = opool.tile([D + 1, 512], F32, tag="o", name="o_b")
                    for g in range(KC):
                        e_t = epool.tile([128, 1024], F32)
                        if pair_qk:
                            nc.tensor.matmul(
                                e_t[:, 0:512],
                                lhsT=(kt_sb[p][0:D, g * 128 : (g + 1) * 128]),
                                rhs=(qs_lo),
                                start=True,
                                stop=True,
                            )
                            nc.tensor.matmul(
                                e_t[:, 512:1024],
                                lhsT=(kt_sb[p][D : 2 * D, g * 128 : (g + 1) * 128]),
                                rhs=(qs_hi),
                                start=True,
                                stop=True,
                            )
                        else:
                            for hh in range(2):
                                nc.tensor.matmul(
                                    e_t[:, hh * 512 : (hh + 1) * 512],
                                    lhsT=(
                                        kh_sb[2 * p + hh][
                                            :, g * 128 : (g + 1) * 128
                                        ]
                                    ),
                                    rhs=(
                                        qh_sb[2 * p + hh][
                                            :, qb * 512 : (qb + 1) * 512
                                        ]
                                    ),
                                    start=True,
                                    stop=True,
                                )
                        a_t = apool.tile([128, 1024], BF16)
                        if exp_idx % 16 in dve_slots:
                            nc.vector._custom_dve(
                                exp_op, out=a_t, in0=e_t, s0=B2, s1=B1, imm2=B0
                            )
                        else:
                            nc.scalar.activation(
                                out=a_t,
                                in_=e_t,
                                func=mybir.ActivationFunctionType.Exp,
                                scale=ACT_SCALE,
                            )
                        exp_idx += 1
                        if debug_x and qb == 0 and p == 0 and g == 0:
                            e_dump = singles.tile([128, 1024], F32, tag="edump")
                            nc.vector.tensor_copy(e_dump, e_t)
                            nc.sync.dma_start(out=edbg[:, :], in_=e_dump)
                            nc.sync.dma_start(out=adbg[:, :], in_=a_t)
                        if pend_att is not None:
                            pend_att()
                        pend_att = make_att(p, a_t, o_a, o_b, g)
                        # deferred normalization from the previous superstep
                        if g == 4 and pend_recip is not None:
                            pend_recip()
                            pend_recip = None
                        if g == 8 and pend_mul is not None:
                            pend_mul()
                            pend_mul = None
                    box = []
                    rbox = []
                    pend_recip = make_recip(o_a, o_b, box, rbox)
                    pend_mul = make_mul(p, qb, o_a, o_b, box)
                    dbg_o = o_a
                    dbg_box = box
                    dbg_rbox = rbox
            # flush tail
            if pend_att is not None:
                pend_att()
                pend_att = None
            if pend_recip is not None:
                pend_recip()
                pend_recip = None
            if pend_mul is not None:
                pend_mul()
                pend_mul = None
            if debug_x:
                o_dump = singles.tile([D + 1, 512], F32, tag="odump")
                nc.vector.tensor_copy(o_dump, dbg_o)
                nc.sync.dma_start(out=odbg[:, :], in_=o_dump)
                nc.sync.dma_start(out=bdbg[:, :], in_=dbg_box[0])
                nc.sync.dma_start(out=rdbg[:, :], in_=dbg_rbox[0][0:1, :])
                for p in range(NP):
                    nc.sync.dma_start(
                        out=xdbg[p * 128 : (p + 1) * 128, :], in_=xt_sb[p]
                    )
            # fc phase: y[q, f] partial, K=128 per pair-stacked xt
            fc_ci = 0
            for q128 in range(S // 128):
                for f in range(E // 512):
                    make_fc(q128, f, fc_ci)()
                    fc_ci += 1
    nc.compile()
    return nc


class SpmdRunner:
    """Build one jitted shard_map callable over 8 cores; reusable for timing."""

    def __init__(self, nc, n_cores):
        import jax
        from jax.experimental.shard_map import shard_map
        from jax.sharding import Mesh, PartitionSpec

        from concourse import mybir
        from concourse.bass2jax import _bass_exec_p, install_neuronx_cc_hook
        from concourse.bass2jax import partition_id_tensor as _pid

        install_neuronx_cc_hook()
        self.jax = jax
        self.nc = nc
        self.n_cores = n_cores
        self.PartitionSpec = PartitionSpec

        partition_name = nc.partition_id_tensor.name if nc.partition_id_tensor else None
        in_names, out_names, out_avals = [], [], []
        for alloc in nc.m.functions[0].allocations:
            if not isinstance(alloc, mybir.MemoryLocationSet):
                continue
            name = alloc.memorylocations[0].name
            if alloc.kind == "ExternalInput":
                if name != partition_name:
                    in_names.append(name)
            elif alloc.kind == "ExternalOutput":
                out_names.append(name)
                shape = tuple(alloc.tensor_shape)
                dtype = mybir.dt.np(alloc.dtype)
                out_avals.append(jax.core.ShapedArray(shape, dtype))
        self.in_names = in_names
        self.out_names = out_names
        self.out_avals = out_avals
        n_params = len(in_names)
        n_outs = len(out_avals)

        all_in_names = list(in_names) + list(out_names)
        if partition_name is not None:
            all_in_names.append(partition_name)

        def _body(*args):
            operands = list(args)
            if partition_name is not None:
                operands.append(_pid())
            outs = _bass_exec_p.bind(
                *operands,
                out_avals=tuple(out_avals),
                in_names=tuple(all_in_names),
                out_names=tuple(out_names),
                lowering_input_output_aliases=(),
                sim_require_finite=True,
                sim_require_nnan=True,
                nc=nc,
            )
            return tuple(outs)

        self._body = _body
        devices = jax.devices()[:n_cores]
        assert len(devices) == n_cores
        self.mesh = Mesh(np.asarray(devices), ("core",))
        in_specs = (PartitionSpec("core"),) * (n_params + n_outs)
        out_specs = (PartitionSpec("core"),) * n_outs
        self.fn = jax.jit(
            shard_map(
                _body,
                mesh=self.mesh,
                in_specs=in_specs,
                out_specs=out_specs,
                check_rep=False,
            ),
            keep_unused=True,
        )
        self._chain_fns = {}

    def prepare(self, in_maps):
        jax = self.jax
        n = self.n_cores
        concat_in = [
            np.concatenate([np.asarray(in_maps[c][name]) for c in range(n)], axis=0)
            for name in self.in_names
        ]
        concat_zeros = [
            np.zeros((n * a.shape[0], *a.shape[1:]), a.dtype) for a in self.out_avals
        ]
        sharding = jax.sharding.NamedSharding(self.mesh, self.PartitionSpec("core"))
        self.dev_args = [jax.device_put(a, sharding) for a in concat_in + concat_zeros]
        return self.dev_args

    def run(self):
        outs = self.fn(*self.dev_args)
        self.jax.block_until_ready(outs)
        return outs

    def results(self, outs):
        n = self.n_cores
        res = []
        for c in range(n):
            d = {}
            for i, name in enumerate(self.out_names):
                a = np.asarray(outs[i])
                d[name] = a.reshape(n, *self.out_avals[i].shape)[c]
            res.append(d)
        return res

    # ---- timing support: chain K invocations through the tok tensor ----
    def chain_fn(self, k):
        if k in self._chain_fns:
            return self._chain_fns[k]
        jax = self.jax
        from jax.experimental.shard_map import shard_map

        tok_in_idx = self.in_names.index("tok")
        tok_out_idx = self.out_names.index("tok_out")
        n_params = len(self.in_names)

        def _chained(*args):
            args = list(args)
            outs = None
            for _ in range(k):
                outs = self._body(*args)
                args[tok_in_idx] = outs[tok_out_idx]
            return tuple(outs)

        in_specs = (self.PartitionSpec("core"),) * (n_params + len(self.out_names))
        out_specs = (self.PartitionSpec("core"),) * len(self.out_names)
        fn = jax.jit(
            shard_map(
                _chained,
                mesh=self.mesh,
                in_specs=in_specs,
                out_specs=out_specs,
                check_rep=False,
            ),
            keep_unused=True,
        )
        self._chain_fns[k] = fn
        return fn

    def time_chain(self, k, iters=8, warmup=2):
        fn = self.chain_fn(k)
        for _ in range(warmup):
            self.jax.block_until_ready(fn(*self.dev_args))
        ts = []
        for _ in range(iters):
            t0 = time.perf_counter()
            self.jax.block_until_ready(fn(*self.dev_args))
            ts.append(time.perf_counter() - t0)
        return min(ts)


def shard_inputs(values, keys, query, W_out):
    """Build the 8 per-core input maps (host-side layout prep)."""
    import ml_dtypes

    BF = np.dtype(ml_dtypes.bfloat16)
    v4 = np.asarray(values, np.float32).reshape(NB, S, H, D)
    k4 = np.asarray(keys, np.float32).reshape(NB, S, H, D)
    q4 = np.asarray(query, np.float32).reshape(NB, S, H, D)
    W_out = np.asarray(W_out, np.float32)
    in_maps = []
    tok = np.zeros((1, 128), np.float32)
    for c in range(N_CORES):
        n = c // 4
        h0 = HPC * (c % 4)
        # [NP, 128, S]: rows 0-63 head (h0+2p), rows 64-127 head (h0+2p+1)
        qt = np.ascontiguousarray(
            (q4[n, :, h0 : h0 + HPC, :].transpose(1, 2, 0) * np.float32(S_Q))
            .reshape(NP, 128, S)
            .astype(BF)
        )
        kt = np.ascontiguousarray(
            k4[n, :, h0 : h0 + HPC, :].transpose(1, 2, 0).reshape(NP, 128, S)
            .astype(BF)
        )
        vb = np.concatenate(
            [
                np.ascontiguousarray(v4[n, :, h0 : h0 + HPC, :].transpose(1, 0, 2)),
                np.ones((HPC, S, 1), np.float32),
            ],
            axis=2,
        ).astype(BF)  # [HPC, S, D+1]
        wt = np.ascontiguousarray(
            W_out[:, (h0 * D) : (h0 + HPC) * D].T.reshape(NP, 128, E).astype(BF)
        )
        in_maps.append({"qt": qt, "kt": kt, "vb": vb, "wt": wt, "tok": tok})
    return in_maps


_CACHE = {}


def get_runner():
    if "runner" not in _CACHE:
        nc = build_kernel()
        _CACHE["runner"] = SpmdRunner(nc, N_CORES)
    return _CACHE["runner"]


def kernel(values, keys, query, W_out, b_out):
    runner = get_runner()
    in_maps = shard_inputs(values, keys, query, W_out)
    runner.prepare(in_maps)
    outs = runner.run()
    res = runner.results(outs)
    y = np.zeros((NB, S, E), np.float32)
    for c in range(N_CORES):
        y[c // 4] += res[c]["yp"].astype(np.float32)
    y += np.asarray(b_out, np.float32)[None, None, :]
    return y
